# revision 8
# baseline (speedup 1.0000x reference)
"""BiLSTM model kernel for 8 Trainium2 NeuronCores.

Model (matches reference): e = emb[x]; h_f = LSTM_fwd(e)[-1]; h_b = LSTM_bwd(e)[-1];
out = sigmoid(concat(h_f, h_b) @ fc_w.T + fc_b).

Sharding: 8 cores = 4 batch shards (64 rows each) x 2 directions. Every core runs
the identical SPMD program: a 512-step LSTM scan for one direction over its
batch shard. The backward direction is realized by feeding the time-reversed
token sequence.

The scan is latency-bound on the recurrent chain, so the step is structured to
minimize serial engine hops (each cross-engine hop costs ~100-200 ns in sem
propagation + SBUF access latency):
  - all matmuls in bf16 (1 cycle/row vs 4 for fp32); PSUM accumulates fp32
  - gate pre-activations in COLUMN blocks [f | i | 2g] of one PSUM bank and
    [o] in a second bank, so ONE sigmoid covers the critical three gates and
    only waits on their three h-proj matmuls; sigma(o) runs in ACT idle time
  - tanh(g) is never computed: with the g-weights pre-scaled by 2,
    tanh(g) = 2*sigma(2g) - 1, folded into a fused scalar_tensor_tensor:
      P = (sg - 0.5) * si;  c' = 2*P + (sf * c)
    which removes an ACT visit (+ its ~500 ns latency) from the chain
  - bias rides as a constant ones-row in the persistent h tile (K=65 h-proj),
    so every step is uniform (h0 = 0 still yields gates = W e + b)
  - per-step chain: MM(3) -> sigmoid -> DVE(P, Q, c') -> tanh -> DVE(h) -> MM

The embedding lookup runs on-device (indirect-DMA row gathers in bf16 + PE
transpose), pipelined ahead of the scan; the PSUM->SBUF chunk copies run on
the otherwise-idle Pool engine so they never touch the chain's engines.
"""

import sys

sys.path.insert(0, "/opt/trn_rl_repo")

import numpy as np
import ml_dtypes

import concourse.bacc as bacc
import concourse.bass as bass
import concourse.mybir as mybir
import concourse.tile as tile
from concourse.bass_utils import run_bass_kernel_spmd

F32 = mybir.dt.float32
BF16 = mybir.dt.bfloat16
AF = mybir.ActivationFunctionType
ALU = mybir.AluOpType

V, E, HID, B, S = 50000, 100, 64, 256, 512
N_CORES = 8
BC = B // 4  # 64 batch rows per core; cores 0-3 forward, 4-7 backward
KU = HID + 1  # h-proj contraction: hidden dims + ones row (bias)

_built = {}


def _build(s_len=S, bc=BC, repeats=1, gather=True):
    """Build + compile the single SPMD program (one LSTM direction scan).

    repeats > 1 runs the whole scan (including the gather pipeline) that many
    times with state reset in between; (T(R)-T(1))/(R-1) isolates one full
    512-step scan + overlapped gather free of dispatch overhead."""
    key = (s_len, bc, repeats, gather)
    if key in _built:
        return _built[key]

    nc = bacc.Bacc("TRN2", target_bir_lowering=False, debug=False, num_devices=N_CORES)

    n_tok = s_len * bc
    n_chunks = (n_tok + 127) // 128
    emb_d = nc.dram_tensor("emb", [V, E], BF16, kind="ExternalInput")
    idx_d = nc.dram_tensor("idx", [128, n_chunks], mybir.dt.int32,
                           kind="ExternalInput")
    # gate column blocks: f | i | 2g | o (g pre-scaled by 2 for the
    # tanh(g) = 2*sigmoid(2g)-1 identity); bias in u_all row 64
    w_all = nc.dram_tensor("w_all", [E, 256], BF16, kind="ExternalInput")
    u_all = nc.dram_tensor("u_all", [KU, 256], BF16, kind="ExternalInput")
    y = nc.dram_tensor("y", [HID, bc], BF16, kind="ExternalOutput")

    with tile.TileContext(nc) as tc:
        with (
            tc.tile_pool(name="const", bufs=1) as cpool,
            tc.tile_pool(name="state", bufs=1) as spool,
            tc.tile_pool(name="step", bufs=4) as pool,
            tc.tile_pool(name="gath", bufs=10) as gpool,
            tc.tile_pool(name="psumA", bufs=3, space="PSUM") as ppA,
            tc.tile_pool(name="psumB", bufs=2, space="PSUM") as ppB,
        ):
            # 128 partitions: rows 0:E hold embedding dims, rows E:128 are
            # dead padding so the XBAR DMA transpose ([p%16, free%128]) fits
            eT_sb = cpool.tile([128, n_tok], BF16)
            idx_sb = cpool.tile([128, n_chunks], mybir.dt.int32)
            nc.sync.dma_start(out=idx_sb[:], in_=idx_d[:])
            w_sb = cpool.tile([E, 256], BF16)
            nc.sync.dma_start(out=w_sb[:], in_=w_all[:])
            u_sb = cpool.tile([KU, 256], BF16)
            nc.sync.dma_start(out=u_sb[:], in_=u_all[:])

            C = spool.tile([HID, bc], F32)  # cell state
            H = spool.tile([KU, bc], BF16)  # h state; row 64 = constant 1.0

            # Pre-zero the dead columns of every gather buffer once so the
            # transpose never moves uninitialized (possibly non-finite) bits.
            for _i in range(10):
                Rz = gpool.tile([128, 128], BF16, tag="R")
                nc.gpsimd.memset(Rz[:, E:128], 0.0)

            def gather_chunk(c):
                """Gather 128 embedding rows for chunk c and land them
                transposed in eT_sb[:, c*128:(c+1)*128]. The transpose runs
                on the DMA XBAR, so the gather never touches a compute
                engine that the recurrent chain uses."""
                R = gpool.tile([128, 128], BF16, tag="R")
                nc.gpsimd.indirect_dma_start(
                    out=R[:, 0:E],
                    out_offset=None,
                    in_=emb_d[:],
                    in_offset=bass.IndirectOffsetOnAxis(
                        ap=idx_sb[:, c : c + 1], axis=0
                    ),
                )
                nc.sync.dma_start_transpose(
                    out=eT_sb[:, c * 128 : (c + 1) * 128], in_=R[:]
                )

            def step(t):
                PA = ppA.tile([HID, 3 * bc], F32, tag="PA")  # f | i | 2g
                PB = ppB.tile([HID, bc], F32, tag="PB")  # o
                ecol = eT_sb[0:E, t * bc : (t + 1) * bc]

                # e-projections (off the recurrent chain; start=True
                # pending-zeroes the bank so blocks overwrite-on-first-write)
                for q in range(3):
                    nc.tensor.matmul(
                        PA[:, q * bc : (q + 1) * bc],
                        lhsT=w_sb[:, q * 64 : (q + 1) * 64],
                        rhs=ecol,
                        start=(q == 0),
                        stop=False,
                    )
                nc.tensor.matmul(
                    PB[:], lhsT=w_sb[:, 192:256], rhs=ecol, start=True, stop=False
                )
                # h-projections (on the chain); bank A stops first so the
                # critical sigmoid never waits on the o-gate matmul
                for q in range(3):
                    nc.tensor.matmul(
                        PA[:, q * bc : (q + 1) * bc],
                        lhsT=u_sb[:, q * 64 : (q + 1) * 64],
                        rhs=H[:],
                        start=False,
                        stop=(q == 2),
                    )
                nc.tensor.matmul(
                    PB[:], lhsT=u_sb[:, 192:256], rhs=H[:], start=False, stop=True
                )

                X3 = pool.tile([HID, 3 * bc], BF16, tag="X3")  # sf | si | sg
                nc.scalar.activation(X3[:], PA[:], AF.Sigmoid)
                SO = pool.tile([HID, bc], BF16, tag="SO")
                nc.scalar.activation(SO[:], PB[:], AF.Sigmoid)

                Pt = pool.tile([HID, bc], BF16, tag="P")
                nc.vector.scalar_tensor_tensor(  # (sg - 0.5) * si
                    out=Pt[:], in0=X3[:, 2 * bc : 3 * bc], scalar=0.5,
                    in1=X3[:, bc : 2 * bc], op0=ALU.subtract, op1=ALU.mult,
                )
                Qt = pool.tile([HID, bc], F32, tag="Q")
                nc.vector.tensor_tensor(  # sf * c
                    out=Qt[:], in0=X3[:, 0:bc], in1=C[:], op=ALU.mult
                )
                nc.vector.scalar_tensor_tensor(  # c' = 2*P + Q (in place)
                    out=C[:], in0=Pt[:], scalar=2.0, in1=Qt[:],
                    op0=ALU.mult, op1=ALU.add,
                )
                TC = pool.tile([HID, bc], BF16, tag="TC")
                nc.scalar.activation(TC[:], C[:], AF.Tanh)
                nc.vector.tensor_tensor(  # h = so * tanh(c')
                    out=H[0:HID, :], in0=SO[:], in1=TC[:], op=ALU.mult
                )

            PF = 8  # chunks of gather prefetch ahead of the scan
            for _rep in range(repeats):
                nc.vector.memset(C[:], 0.0)
                nc.vector.memset(H[0:HID, :], 0.0)
                nc.vector.memset(H[HID : HID + 1, :], 1.0)
                for c in range(min(PF, n_chunks)):
                    gather_chunk(c)
                for t in range(s_len):
                    if t % 2 == 0:
                        c = t // 2 + PF
                        if c < n_chunks:
                            gather_chunk(c)
                    step(t)

            nc.sync.dma_start(out=y[:], in_=H[0:HID, :])

    nc.compile()
    _built[key] = nc
    return nc


def _pack_weights(W_ih, W_hh, b_ih, b_hh):
    """Host-side packing for one direction: column blocks f | i | 2g | o,
    g-block scaled by 2; w_all [E, 256] bf16, u_all [65, 256] bf16 with the
    bias in row 64."""
    b = (b_ih + b_hh).astype(np.float32)
    order = [1, 0, 2, 3]  # blocks f,i,g,o <- reference gate rows i,f,g,o
    W4 = np.concatenate([W_ih[q * HID : (q + 1) * HID] for q in order], axis=0)
    U4 = np.concatenate([W_hh[q * HID : (q + 1) * HID] for q in order], axis=0)
    b4 = np.concatenate([b[q * HID : (q + 1) * HID] for q in order])
    W4 = W4.copy(); U4 = U4.copy(); b4 = b4.copy()
    W4[2 * HID : 3 * HID] *= 2.0
    U4[2 * HID : 3 * HID] *= 2.0
    b4[2 * HID : 3 * HID] *= 2.0
    w_all = np.ascontiguousarray(W4.T).astype(ml_dtypes.bfloat16)
    u_all = np.ascontiguousarray(
        np.concatenate([U4.T, b4[None, :]], axis=0)
    ).astype(ml_dtypes.bfloat16)
    return w_all, u_all


def _prepare_in_maps(inputs, s_len=S, bc=BC, gather=True):
    x = np.asarray(inputs["x"])
    emb = np.asarray(inputs["emb"], dtype=np.float32).astype(ml_dtypes.bfloat16)
    emb = np.ascontiguousarray(emb)
    pk_f = _pack_weights(
        np.asarray(inputs["W_ih_f"], np.float32), np.asarray(inputs["W_hh_f"], np.float32),
        np.asarray(inputs["b_ih_f"], np.float32), np.asarray(inputs["b_hh_f"], np.float32),
    )
    pk_b = _pack_weights(
        np.asarray(inputs["W_ih_b"], np.float32), np.asarray(inputs["W_hh_b"], np.float32),
        np.asarray(inputs["b_ih_b"], np.float32), np.asarray(inputs["b_hh_b"], np.float32),
    )

    batch = x.shape[0]
    n_shards = batch // bc

    in_maps = []
    for core in range(N_CORES):
        fwd = core < n_shards
        shard = core % n_shards
        xs = x[shard * bc : (shard + 1) * bc, :s_len]  # [bc, s]
        if not fwd:
            xs = xs[:, ::-1]
        w_all, u_all = pk_f if fwd else pk_b
        # token j = t*bc + b -> emb row x[b, t]; idx[p, c] covers j = c*128+p
        tok = np.ascontiguousarray(xs.T.reshape(-1).astype(np.int32))  # [n_tok]
        m = {
            "w_all": w_all,
            "u_all": u_all,
            "idx": np.ascontiguousarray(tok.reshape(-1, 128).T),
            "emb": emb,
        }
        in_maps.append(m)
    return in_maps


def _postprocess(results, inputs, bc=BC):
    fc_w = np.asarray(inputs["fc_w"], dtype=np.float32)
    fc_b = np.asarray(inputs["fc_b"], dtype=np.float32)
    n_shards = np.asarray(inputs["x"]).shape[0] // bc
    h_f = np.concatenate(
        [np.asarray(results[c]["y"], dtype=np.float32).T for c in range(n_shards)],
        axis=0,
    )
    h_b = np.concatenate(
        [
            np.asarray(results[n_shards + c]["y"], dtype=np.float32).T
            for c in range(n_shards)
        ],
        axis=0,
    )
    h_cat = np.concatenate([h_f, h_b], axis=1)  # [B, 2H]
    out = 1.0 / (1.0 + np.exp(-(h_cat @ fc_w.T + fc_b)))
    return out.astype(np.float32)


def kernel(x, emb, W_ih_f, W_hh_f, b_ih_f, b_hh_f, W_ih_b, W_hh_b, b_ih_b, b_hh_b,
           fc_w, fc_b, s_len=S, bc=BC, gather=True):
    inputs = dict(
        x=x, emb=emb, W_ih_f=W_ih_f, W_hh_f=W_hh_f, b_ih_f=b_ih_f, b_hh_f=b_hh_f,
        W_ih_b=W_ih_b, W_hh_b=W_hh_b, b_ih_b=b_ih_b, b_hh_b=b_hh_b,
        fc_w=fc_w, fc_b=fc_b,
    )
    nc = _build(s_len, bc, gather=gather)
    in_maps = _prepare_in_maps(inputs, s_len, bc, gather=gather)
    res = run_bass_kernel_spmd(nc, in_maps, list(range(N_CORES)))
    return _postprocess(res.results, inputs, bc)
